# revision 21
# baseline (speedup 1.0000x reference)
"""FCOS head (nn_FCOS_73787538145418) Trainium2 Bass kernel — bf16 v2.

Sharding: data-parallel, one image per NeuronCore (B=8 across 8 cores),
weights replicated. Each core runs the identical SPMD NEFF over its image.

Per level (p3 64x64, p4 32x32, p5 16x16): two 4-layer 3x3 conv stems
(cls/box, 256ch + ReLU) and prediction convs (cls 20ch; box+ctr 5ch).

Everything flows in bf16 (features, stem/pred weights, stem activations)
with f32 PSUM accumulation: bf16 matmuls stream 1 row/cycle with fully
pipelined weight loads (fp32r pays ~13ns/matmul extra), and rel err vs the
f32 reference is ~6e-3 — inside the 2e-2 gate.

Structure per core:
- 4 padded SBUF image buffers (each holds all 3 levels). cls chain
  v0->v1->v2->v1->v2 (tower v2); box chain v0->v3->v0->v3->v0 (tower v0).
  cls/box layers interleave so one stem's trailing activations always hide
  under the other stem's matmuls.
- Stem layer = 18 PSUM-accumulated matmuls (2 cin chunks x 9 taps) per
  (cout chunk, row tile) over padded views; all 3 levels share each layer's
  weight DMA. p3's 8 row-tiles run in halves of 4 so PSUM banks recycle
  without stalling the PE.
- Prediction convs are kx-stacked: lhsT packs 3 kx taps x channels into
  32-aligned partition groups, ky+cin accumulate in PSUM (6 matmuls per
  tile per chain instead of 18); the tap combine is 1 scalar-engine
  Identity+bias op plus 2 vector adds reading shifted PSUM windows.
- First cls layer on p3 runs tile-major so compute starts after the first
  feature band + half the first weight tile arrive (~5us) instead of the
  whole image.

Output is [25, 5376] channel-major per core; host transposes and stacks.
"""
import sys

if '/opt/trn_rl_repo' not in sys.path:
    sys.path.insert(0, '/opt/trn_rl_repo')

import numpy as np
import ml_dtypes

import concourse.mybir as mybir
from concourse import bacc
import concourse.tile as tile
from concourse.bass_utils import run_bass_kernel_spmd

P = 128
NCH = 2                 # 256 channels = 2 chunks of 128
NL = 4                  # stem depth
BF16 = mybir.dt.bfloat16
F32 = mybir.dt.float32
RELU = mybir.ActivationFunctionType.Relu
IDENT = mybir.ActivationFunctionType.Identity
ADD = mybir.AluOpType.add

# (H, W, flat pixel base in the packed output)
LEVELS = [(64, 64, 0), (32, 32, 4096), (16, 16, 5120)]
N3 = NCH * 66 * 66      # 8712 padded elems/partition
N4 = NCH * 34 * 34      # 2312
N5 = NCH * 18 * 18      # 648
PAD_N = N3 + N4 + N5    # 11672
NPIX = 5376

# buffer chain: cls v0->v1->v2->v1->v2 ; box v0->v3->v0->v3->v0
CHAIN = [[(0, 1), (1, 2), (2, 1), (1, 2)],
         [(0, 3), (3, 0), (0, 3), (3, 0)]]

# pred row tiles per level: (r0, R) with (R+0)*(W+2) <= 512 psum floats
PRED_TILES = [
    [(r, 7) for r in range(0, 56, 7)] + [(56, 4), (60, 4)],   # p3
    [(0, 11), (11, 11), (22, 10)],                            # p4
    [(0, 16)],                                                # p5
]

_cached = {}
_run_opts = {}   # extra kwargs for run_bass_kernel_spmd (test harness: trace)
_last = {}       # last BassKernelResults (test harness reads exec_time_ns)


def _views(t, lvl):
    off = [0, N3, N3 + N4][lvl]
    H, W, _ = LEVELS[lvl]
    n = NCH * (H + 2) * (W + 2)
    return t[:, off:off + n].rearrange("p (c h w) -> p c h w",
                                       c=NCH, h=H + 2, w=W + 2)


def _zero_ring(nc, v, H, W):
    for c in range(NCH):
        nc.vector.memset(v[:, c, 0, :], 0.0)
        nc.vector.memset(v[:, c, H + 1, :], 0.0)
        nc.vector.memset(v[:, c, 1:H + 1, 0], 0.0)
        nc.vector.memset(v[:, c, 1:H + 1, W + 1], 0.0)


def _stem_level(nc, psum_pool, wto, src, dst, bias, o, H, W, R, tiles, tag,
                src_r0=0):
    """One level of one stem conv layer for one cout chunk.

    wto: this cout chunk's weight tile [P, c, tap, P]. tiles: row-tile
    indices; src_r0: offset of src's first padded row (for split feat
    tiles). Emits len(tiles)*18 matmuls + len(tiles) activations."""
    def rhs(c, it, ky, kx):
        r0 = it * R - src_r0
        return src[:, c, r0 + ky:r0 + ky + R, kx:kx + W]

    pss = {it: psum_pool.tile([P, R, W], F32, tag="ps",
                              name=f"ps_{tag}_{it}") for it in tiles}
    k = 0
    for c in range(NCH):
        for t in range(9):
            ky, kx = divmod(t, 3)
            for it in tiles:
                nc.tensor.matmul(pss[it][:], wto[:, c, t], rhs(c, it, ky, kx),
                                 start=(k == 0), stop=(k == 17))
            k += 1
    for it in tiles:
        r0 = it * R
        nc.scalar.activation(dst[:, o, r0 + 1:r0 + 1 + R, 1:W + 1],
                             pss[it][:], RELU, bias=bias)


def _stem_layer(nc, psum_pool, vs, wts, sbias, s, l, feat_p3=None):
    """wts: per-cout-chunk weight tiles (wt_o0, wt_o1). feat_p3: for the
    first layer, (fa, fb) split feature tiles replacing vs[si][0]."""
    si, di = CHAIN[s][l]
    for o in range(NCH):
        bias = sbias[:, s, l, o]
        tag = f"{s}{l}{o}"
        if feat_p3 is not None:
            fa, fb = feat_p3
            if s == 0 and o == 0:
                # startup: tile-pairs with c-major inner loops so the
                # first 18 matmuls consume only the first-arriving
                # (c0) feature/weight halves
                halves = ([0, 1], [2, 3], [4, 5], [6, 7])
            else:
                halves = ([0, 1, 2, 3], [4, 5, 6, 7])
            for tiles in halves:
                f, r0 = (fa, 0) if tiles[0] < 4 else (fb, 32)
                _stem_level(nc, psum_pool, wts[o], f, vs[di][0], bias,
                            o, 64, 64, 8, tiles, f"{tag}h{tiles[0]}",
                            src_r0=r0)
        else:
            _stem_level(nc, psum_pool, wts[o], vs[si][0], vs[di][0], bias, o,
                        64, 64, 8, [0, 1, 2, 3], tag + "a")
            _stem_level(nc, psum_pool, wts[o], vs[si][0], vs[di][0], bias, o,
                        64, 64, 8, [4, 5, 6, 7], tag + "b")
        # p4 (2 tiles) + p5 (1 tile) share the weight groups' locality
        _stem_level(nc, psum_pool, wts[o], vs[si][1], vs[di][1], bias, o,
                    32, 32, 16, [0, 1], tag + "p4")
        _stem_level(nc, psum_pool, wts[o], vs[si][2], vs[di][2], bias, o,
                    16, 16, 16, [0], tag + "p5")


def _pred_tile(nc, psum_pool, stage_pool, pwc, pwb, pbc, pbb,
               tcls, tbox, out_d, lvl, r0, R, tag):
    H, W, pix = LEVELS[lvl]
    Wp = W + 2
    psc = psum_pool.tile([96, R, Wp], F32, tag="ps", name=f"pc_{tag}")
    psb = psum_pool.tile([96, R, Wp], F32, tag="ps", name=f"pb_{tag}")
    k = 0
    for c in range(NCH):
        for ky in range(3):
            nc.tensor.matmul(psc[0:96], pwc[:, c, ky],
                             tcls[:, c, r0 + ky:r0 + ky + R, 0:Wp],
                             start=(k == 0), stop=(k == 5))
            nc.tensor.matmul(psb[0:96], pwb[:, c, ky],
                             tbox[:, c, r0 + ky:r0 + ky + R, 0:Wp],
                             start=(k == 0), stop=(k == 5))
            k += 1
    stc = stage_pool.tile([32, R, W], F32, tag="st", name=f"sc_{tag}")
    stb = stage_pool.tile([32, R, W], F32, tag="st", name=f"sb_{tag}")
    # out[ch] = Z[kx0, w] + Z[kx1, w+1] + Z[kx2, w+2] + bias ; kx groups
    # live at 32-aligned psum partitions (HW: one PSUM operand per op)
    nc.scalar.activation(stc[0:20], psc[32:52, :, 1:W + 1], IDENT,
                         bias=pbc[32:52])
    nc.vector.tensor_tensor(stc[0:20], psc[0:20, :, 0:W], stc[0:20], ADD)
    nc.vector.tensor_tensor(stc[0:20], psc[64:84, :, 2:W + 2], stc[0:20], ADD)
    nc.scalar.activation(stb[0:5], psb[32:37, :, 1:W + 1], IDENT,
                         bias=pbb[32:37])
    nc.vector.tensor_tensor(stb[0:5], psb[0:5, :, 0:W], stb[0:5], ADD)
    nc.vector.tensor_tensor(stb[0:5], psb[64:69, :, 2:W + 2], stb[0:5], ADD)
    c0 = pix + r0 * W
    nc.gpsimd.dma_start(out_d[0:20, c0:c0 + R * W],
                        stc[0:20].rearrange("p r w -> p (r w)"))
    nc.gpsimd.dma_start(out_d[20:25, c0:c0 + R * W],
                        stb[0:5].rearrange("p r w -> p (r w)"))


def _build():
    nc = bacc.Bacc("TRN2", target_bir_lowering=False, debug=False,
                   num_devices=8)

    x_d = [nc.dram_tensor(f"x{i}", (NCH, P, H + 2, W + 2), BF16,
                          kind="ExternalInput")
           for i, (H, W, _) in enumerate(LEVELS)]
    sw_d = nc.dram_tensor("sw", (8, NCH, P, NCH, 9, P), BF16,
                          kind="ExternalInput")
    sb_d = nc.dram_tensor("sb", (2, NL, NCH, P, 1), F32, kind="ExternalInput")
    pwc_d = nc.dram_tensor("pwc", (NCH, 3, P, 96), BF16, kind="ExternalInput")
    pwb_d = nc.dram_tensor("pwb", (NCH, 3, P, 96), BF16, kind="ExternalInput")
    pbc_d = nc.dram_tensor("pbc", (96, 1), F32, kind="ExternalInput")
    pbb_d = nc.dram_tensor("pbb", (96, 1), F32, kind="ExternalInput")
    out_d = nc.dram_tensor("out", (25, NPIX), F32, kind="ExternalOutput")

    with tile.TileContext(nc) as tc:
        with (
            tc.tile_pool(name="resident", bufs=1) as res_pool,
            tc.tile_pool(name="wts", bufs=6) as wts_pool,
            tc.tile_pool(name="psum", bufs=8, space="PSUM") as psum_pool,
            tc.tile_pool(name="stage", bufs=4) as stage_pool,
        ):
            pads = [res_pool.tile([P, PAD_N], BF16, name=f"pad{i}")
                    for i in range(4)]
            vs = [[_views(t, lvl) for lvl in range(3)] for t in pads]
            # p3 feature halves in dedicated tiles: tile-granular write
            # deps mean the first matmuls would otherwise wait for the
            # whole image (+ everything else landing in the same tile)
            fa = res_pool.tile([P, NCH, 34, 66], BF16, name="fa")
            fb = res_pool.tile([P, NCH, 34, 66], BF16, name="fb")

            sbias = res_pool.tile([P, 2, NL, NCH, 1], F32, name="sbias")
            pwc = res_pool.tile([P, NCH, 3, 96], BF16, name="pwc")
            pwb = res_pool.tile([P, NCH, 3, 96], BF16, name="pwb")
            pbc = res_pool.tile([96, 1], F32, name="pbc")
            pbb = res_pool.tile([96, 1], F32, name="pbb")

            # --- startup DMAs ---
            # Every engine queue opens only after ~9us of framework
            # preamble, so the startup-critical transfers go on the sync
            # HWDGE queue (fast generation), smallest-first in the order
            # the tile-major first layer consumes them.
            wt0 = [wts_pool.tile([P, NCH, 9, P], BF16, tag="w",
                                 name=f"w_00{o}") for o in range(NCH)]
            nc.sync.dma_start(wt0[0][:, 0], sw_d[0, 0, :, 0])
            nc.sync.dma_start(fa[:, 0], x_d[0][0, :, 0:34])
            nc.sync.dma_start(wt0[0][:, 1], sw_d[0, 0, :, 1])
            nc.sync.dma_start(fa[:, 1], x_d[0][1, :, 0:34])
            nc.sync.dma_start(
                fb[:], x_d[0][:, :, 32:66].rearrange("c p h w -> p c h w"))
            nc.sync.dma_start(wt0[1][:], sw_d[0, 1])
            nc.gpsimd.dma_start(
                sbias[:],
                sb_d[:].rearrange("s l a p o -> p (s l a o)")
                       .rearrange("p (s l a o) -> p s l a o",
                                  s=2, l=NL, a=NCH))
            # small feats + pred consts on the gpsimd SWDGE queue
            for lvl in (1, 2):
                for c in range(NCH):
                    nc.gpsimd.dma_start(vs[0][lvl][:, c], x_d[lvl][c])
            nc.gpsimd.dma_start(pwc[:], pwc_d[:].rearrange("c k p n -> p c k n"))
            nc.gpsimd.dma_start(pwb[:], pwb_d[:].rearrange("c k p n -> p c k n"))
            nc.gpsimd.dma_start(pbc[0:96], pbc_d[:])
            nc.gpsimd.dma_start(pbb[0:96], pbb_d[:])
            # scratch rings must read as 'same' conv zero padding (v0's p3
            # region is box-chain scratch now that the feat lives in fa/fb)
            _zero_ring(nc, vs[0][0], 64, 64)
            for b in (1, 2, 3):
                for lvl, (H, W, _) in enumerate(LEVELS):
                    _zero_ring(nc, vs[b][lvl], H, W)

            # --- stems: interleave cls/box per layer ---
            for l in range(NL):
                for s in range(2):
                    if s == 0 and l == 0:
                        wt = wt0
                    else:
                        wt = [wts_pool.tile([P, NCH, 9, P], BF16,
                                            tag="w", name=f"w_{s}{l}{o}")
                              for o in range(NCH)]
                        for o in range(NCH):
                            nc.sync.dma_start(wt[o][:], sw_d[s * NL + l, o])
                    if s == 1 and l == 3:
                        break
                    _stem_layer(nc, psum_pool, vs, wt, sbias, s, l,
                                feat_p3=(fa, fb) if l == 0 else None)

            # --- final box layer split around the preds so the p3 tap
            # combines (DVE/Scalar) overlap the remaining stem matmuls ---
            si, di = CHAIN[1][3]

            def pred_level(lvl):
                for ti, (r0, R) in enumerate(PRED_TILES[lvl]):
                    _pred_tile(nc, psum_pool, stage_pool, pwc, pwb, pbc, pbb,
                               vs[2][lvl], vs[0][lvl], out_d, lvl, r0, R,
                               f"{lvl}_{ti}")

            for o in range(NCH):
                bias = sbias[:, 1, 3, o]
                _stem_level(nc, psum_pool, wt[o], vs[si][0], vs[di][0], bias,
                            o, 64, 64, 8, [0, 1, 2, 3], f"13{o}a")
                _stem_level(nc, psum_pool, wt[o], vs[si][0], vs[di][0], bias,
                            o, 64, 64, 8, [4, 5, 6, 7], f"13{o}b")
            pred_level(0)
            for o in range(NCH):
                bias = sbias[:, 1, 3, o]
                _stem_level(nc, psum_pool, wt[o], vs[si][1], vs[di][1], bias,
                            o, 32, 32, 16, [0, 1], f"13{o}p4")
                _stem_level(nc, psum_pool, wt[o], vs[si][2], vs[di][2], bias,
                            o, 16, 16, 16, [0], f"13{o}p5")
            pred_level(1)
            pred_level(2)

    nc.compile()
    return nc


def _pack_stem_w(wcls, wbox):
    # [s][co, ci, ky, kx] -> [(s l), o, cip, c, tap, cop]
    w = np.stack([np.asarray(wcls, np.float32),
                  np.asarray(wbox, np.float32)])     # [2,4,256,256,3,3]
    w = w.reshape(2, NL, NCH, P, NCH, P, 3, 3)        # s l o cop c cip ky kx
    w = w.transpose(0, 1, 2, 5, 4, 6, 7, 3)           # s l o cip c ky kx cop
    return np.ascontiguousarray(
        w.reshape(8, NCH, P, NCH, 9, P)).astype(ml_dtypes.bfloat16)


def _pack_pred_w(w):
    # [n, 256, 3, 3] -> [c, ky, cip, kx*32 + ch] (32-aligned kx groups)
    n = w.shape[0]
    w = np.asarray(w, np.float32).reshape(n, NCH, P, 3, 3) \
        .transpose(1, 3, 2, 4, 0)                     # c ky cip kx n
    out = np.zeros((NCH, 3, P, 3, 32), np.float32)
    out[..., :n] = w
    return np.ascontiguousarray(
        out.reshape(NCH, 3, P, 96)).astype(ml_dtypes.bfloat16)


def _pack_pred_b(b):
    out = np.zeros((96, 1), np.float32)
    n = b.shape[0]
    for g in range(3):
        out[g * 32:g * 32 + n, 0] = b
    return out


def kernel(p3, p4, p5, stem_cls_w, stem_cls_b, stem_box_w, stem_box_b,
           pred_cls_w, pred_cls_b, pred_box_w, pred_box_b,
           pred_ctr_w, pred_ctr_b):
    if 'nc' not in _cached:
        _cached['nc'] = _build()
    nc = _cached['nc']

    B = p3.shape[0]
    sw = _pack_stem_w(stem_cls_w, stem_box_w)
    sb = np.ascontiguousarray(
        np.stack([stem_cls_b, stem_box_b]).reshape(2, NL, NCH, P, 1),
        dtype=np.float32)
    pwc = _pack_pred_w(np.asarray(pred_cls_w))
    pwb = _pack_pred_w(np.concatenate([np.asarray(pred_box_w),
                                       np.asarray(pred_ctr_w)], axis=0))
    pbc = _pack_pred_b(np.asarray(pred_cls_b, np.float32))
    pbb = _pack_pred_b(np.concatenate([pred_box_b, pred_ctr_b])
                       .astype(np.float32))

    shared = {"sw": sw, "sb": sb, "pwc": pwc, "pwb": pwb,
              "pbc": pbc, "pbb": pbb}
    xs = [np.asarray(p3, np.float32), np.asarray(p4, np.float32),
          np.asarray(p5, np.float32)]
    in_maps = []
    for b in range(B):
        m = dict(shared)
        for i, x in enumerate(xs):
            xb = x[b].reshape(NCH, P, x.shape[2], x.shape[3]) \
                .astype(ml_dtypes.bfloat16)
            m[f"x{i}"] = np.pad(xb, ((0, 0), (0, 0), (1, 1), (1, 1)))
        in_maps.append(m)

    res = run_bass_kernel_spmd(nc, in_maps, core_ids=list(range(B)),
                               **_run_opts)
    _last['res'] = res
    out = np.stack([np.asarray(r["out"], np.float32).T for r in res.results])
    return np.ascontiguousarray(out, dtype=np.float32)
